# revision 52
# baseline (speedup 1.0000x reference)
"""Trainium2 Bass kernel for nn_DecoderTrans (dense transformer decoder layer + vocab head).

Sharding: 8 cores = (batch b, half hf). Each core computes the full trunk for its
512 "own" tokens (queries) and the K/V context for the whole 1024-token sequence
of its batch element. Own tokens always occupy key slots [512, 1024).

v4: startup overlap (ekT/evsb projections emitted first + PE warmup matmuls),
V-bias folds into Wo biases (host), LN stats matmuls interleaved into the
projection blocks, LN gains folded into consumer weights so cq/W1 matmuls run
on pre-norm activations (correction at PSUM evacuation), LN3 folded into the
vocab head (device exports mu/rstd, host applies the affine dequant).
"""
import math
import sys

sys.path.insert(0, "/opt/trn_rl_repo")

import numpy as np

import concourse.bass as bass
import concourse.tile as tile
from concourse import bacc, mybir
from concourse.bass import ts
from concourse.masks import make_identity

P = 128
D = 512
DC = D // P          # 4 feature chunks
T = 1024             # full sequence (keys)
TOWN = 512           # own tokens per core (queries), slots [512, 1024)
H = 8
DKH = 64             # head dim
V = 32000
VCH = 500            # vocab columns per matmul (fits PSUM bank)
VG = 4               # vocab chunks per group
NVG = V // (VCH * VG)  # 16 groups
GW = VG * VCH        # 2000 vocab cols per group
FFN = 2 * D
FC = FFN // P        # 8 hidden chunks
NEG = -30000.0
SQRT_D = math.sqrt(D)
PAD_ID = 0
NWARM = 28

F32 = mybir.dt.float32
I32 = mybir.dt.int32
BF16 = mybir.dt.bfloat16
AF = mybir.ActivationFunctionType
OP = mybir.AluOpType

MF = BF16
WOUT_DT = BF16
OUT_DT = BF16

# packed bias/scale column layout: name -> (offset, width) in the cols input
_COL_LAYOUT = {}
_off = 0
for _nm, _w in (("bq", DC), ("bk", DC), ("bo1", DC), ("cbq", DC), ("ebk", DC),
                ("bo2", DC), ("b2", DC), ("b1", FC),
                ("gc1", DC), ("bc1", DC), ("gc2", DC), ("bc2", DC),
                ("cs1", DC), ("ws1", FC), ("mvS", 8), ("mvC", 8)):
    _COL_LAYOUT[_nm] = (_off, _w)
    _off += _w
NCOLS = _off

# self-attn segment tables: (seg_width, parts, tris)
# part = (kc, q_off, q_w, dst_off); tri = (dst_off,)
SEGS_CAUSAL = [
    (1024, [(0, 0, 512, 0), (1, 0, 512, 512)], []),
    (1024, [(2, 0, 512, 0), (3, 0, 512, 512)], []),
    (896,  [(4, 0, 512, 0), (5, 128, 384, 512)], [0, 512]),
    (384,  [(6, 256, 256, 0), (7, 384, 128, 256)], [0, 256]),
]
SEGS_FULL = [
    (1024, [(kc, 0, 512, 0), (kc + 1, 0, 512, 512)], []) for kc in (0, 2, 4, 6)
]


def build_module():
    nc = bacc.Bacc("TRN2", target_bir_lowering=False, debug=False)

    def din(name, shape, dt=F32):
        return nc.dram_tensor(name, shape, dt, kind="ExternalInput").ap()

    a = {}
    a["idx"] = din("idx", [T, 1], I32)
    a["emb"] = din("emb", [V, D], BF16)
    a["peT"] = din("peT", [DC, P, T], BF16)
    a["encT"] = din("encT", [DC, P, T], BF16)
    a["tri"] = din("tri", [P, P], BF16)
    a["cols"] = din("cols", [P, NCOLS])
    a["srows"] = din("srows", [1, D + FFN], BF16)
    for nm in ("WqT", "WkT", "WvT", "Wo1T", "cWqT", "eWkT", "eWvT", "Wo2T"):
        a[nm] = din(nm, [D, D], MF)
    a["W1T"] = din("W1T", [D, FFN], MF)
    a["W2T"] = din("W2T", [FFN, D], MF)
    a["WoutT"] = din("WoutT", [D, V], WOUT_DT)
    a["out"] = nc.dram_tensor("out", [TOWN, V], OUT_DT, kind="ExternalOutput").ap()
    a["stats"] = nc.dram_tensor("stats", [2, TOWN], F32,
                                kind="ExternalOutput").ap()

    with tile.TileContext(nc) as tc, \
         nc.allow_low_precision(reason="bf16 matmul operand pipeline"):
        _emit(tc, a)
    nc.compile()
    return nc


def _emit(tc, a):
    nc = tc.nc

    def wload(pool, name, nch, width, tag=None):
        """One batched DMA for a [nch*P, width] DRAM weight -> [P, nch*width]."""
        t = pool.tile([P, nch, width], MF, tag=tag or name, name=name)
        nc.sync.dma_start(
            t[:], a[name].rearrange("(c p) n -> p c n", p=P))
        return t

    with tc.tile_pool(name="const", bufs=1) as cp, \
         tc.tile_pool(name="trunk", bufs=1) as trunkp, \
         tc.tile_pool(name="wLate", bufs=1) as wlp, \
         tc.tile_pool(name="wD", bufs=1) as wDp:
        # Pre-load the ln+exp activation table (act_func_sets[6]): every
        # scalar function used here (Exp/Ln/Square/Copy/Identity/Relu) lives
        # in it, so the compiler's fixpoint inserts no further table loads.
        nc.scalar.add_instruction(
            mybir.InstLoadActFuncSet(
                act_func_set_id=6, name=f"I-{nc.next_id()}",
                engine=mybir.EngineType.Activation))

        # ---- critical-path DMAs first (cols/encT/ewk feed the first
        # real tensor work; idx only gates the gpsimd gathers) ----
        cols = cp.tile([P, NCOLS], F32, tag="cols")
        nc.sync.dma_start(cols[:], a["cols"][:, :])
        idx_sb = cp.tile([P, 8], I32, tag="idx")
        tri = cp.tile([P, P], BF16, tag="tri")
        warm = cp.tile([P, TOWN], BF16, tag="warm")
        srows = cp.tile([1, D + FFN], BF16, tag="srows")
        nc.sync.dma_start(srows[:], a["srows"][:, :])

        def col(nm, i=0, n=1):
            off, w = _COL_LAYOUT[nm]
            assert i + n <= w
            return cols[:, off + i: off + i + n]

        # ---- long-lived trunk activations ----
        x1T = [trunkp.tile([P, TOWN], MF, tag=f"x1T{c}", name=f"x1T{c}")
               for c in range(DC)]
        x2T = [trunkp.tile([P, TOWN], MF, tag=f"x2T{c}", name=f"x2T{c}")
               for c in range(DC)]

        # ---- vocab weight ring (2 groups in flight) ----
        wD_tiles = {}

        def load_wout(vg):
            w = wDp.tile([P, DC, GW], WOUT_DT, tag="wo", bufs=2,
                         name=f"wo{vg}")
            nc.sync.dma_start(
                w[:], a["WoutT"][:, vg * GW:(vg + 1) * GW].rearrange(
                    "(c p) n -> p c n", p=P))
            wD_tiles[vg] = w

        # ================= shared helpers =================

        def proj_fm(dsts, src_halves, w_sb, bias, pp=None, psum_tag="proj"):
            # bias-add evacuation on the scalar engine (idle pre-attention)
            for m in range(len(dsts)):
                for th in range(len(src_halves)):
                    ps = pp.tile([P, 512], F32, tag=psum_tag)
                    for c in range(DC):
                        nc.tensor.matmul(
                            ps[:],
                            lhsT=w_sb[:, c, ts(m, P)],
                            rhs=src_halves[th][c][:, :],
                            start=(c == 0), stop=(c == DC - 1))
                    nc.scalar.activation(
                        dsts[m][:, th * 512:(th + 1) * 512],
                        ps[:], AF.Identity, bias=bias[:, m: m + 1])

        def vproj_unit(vt, src_slice, w_sb, m_col, pp, psum_tag, t):
            """vt [P, H*128]: cols h*128..+63 = V features, +64..+127 = mask
            (mask half pre-filled at startup by mask_fill)."""
            ps = pp.tile([P, D], F32, tag=psum_tag, name="vps")
            for c in range(DC):
                nc.tensor.matmul(ps[:], lhsT=src_slice(c, t), rhs=w_sb[:, c, :],
                                 start=(c == 0), stop=(c == DC - 1))
            v3 = vt[:].rearrange("p (h e) -> p h e", e=P)
            ps3 = ps[:].rearrange("p (h e) -> p h e", e=DKH)
            nc.vector.tensor_scalar(
                v3[:, :, 0:DKH], ps3, m_col, None, op0=OP.mult)

        # ---- attention (pipelined emission; writes mergedT) ----
        def attention(kT, vtiles, qT, causal, mergedT, pools, s_bufs,
                      av_bufs=2, fillers=None):
            sp, avp, sbp = pools
            segs = SEGS_CAUSAL if causal else SEGS_FULL
            pts = {}

            def emit_S(j):
                for half in (0, 1):
                    pts[(j, half)] = []
                for (w, parts, tris) in segs:
                    s = {half: sp.tile([P, 1024], F32, tag="s", bufs=s_bufs,
                                       name=f"s{j}_{half}")
                         for half in (0, 1)}
                    for (kc, qoff, qw, doff) in parts:
                        for half in (0, 1):
                            off = half * DKH
                            nc.tensor.matmul(
                                s[half][:, doff:doff + qw],
                                lhsT=kT[j][off:off + DKH, ts(kc, P)],
                                rhs=qT[j][off:off + DKH, qoff:qoff + qw],
                                start=True, stop=not tris,
                                skip_group_check=True)
                    for doff in tris:
                        for half in (0, 1):
                            nc.tensor.matmul(
                                s[half][:, doff:doff + P],
                                lhsT=ident_b[:], rhs=tri[:],
                                start=False, stop=True,
                                skip_group_check=True)
                    for half in (0, 1):
                        pt = sbp.tile([P, 1024], BF16, tag="pt", bufs=8,
                                      name=f"pt{j}_{half}")
                        nc.scalar.activation(pt[:, 0:w], s[half][:, 0:w],
                                             AF.Exp, scale=0.125)
                        pts[(j, half)].append(pt)

            def emit_AV(j):
                for half in (0, 1):
                    h = 2 * j + half
                    off = half * DKH
                    av = avp.tile([P, TOWN], F32, tag="av", bufs=av_bufs,
                                  name=f"av{j}_{half}")
                    first = True
                    nseg = len(segs)
                    for si, (w, parts, tris) in enumerate(segs):
                        pt = pts[(j, half)][si]
                        for pi, (kc, qoff, qw, doff) in enumerate(parts):
                            last = (si == nseg - 1) and (pi == len(parts) - 1)
                            nc.tensor.matmul(
                                av[:, qoff:qoff + qw],
                                lhsT=vtiles[kc][:, h * P:(h + 1) * P],
                                rhs=pt[:, doff:doff + qw],
                                start=first, stop=last,
                                skip_group_check=True)
                            first = False
                    den = sbp.tile([DKH, TOWN], F32, tag="den", bufs=2,
                                   name=f"den{j}_{half}")
                    nc.vector.tensor_copy(den[:], av[DKH:P, :])
                    rinv = sbp.tile([DKH, TOWN], F32, tag="rinv", bufs=2,
                                    name=f"rinv{j}_{half}")
                    nc.vector.reciprocal_approx_fast(out=rinv[:], in_=den[:])
                    nc.vector.tensor_tensor(
                        mergedT[j][off:off + DKH, :],
                        av[0:DKH, :], rinv[:], op=OP.mult)

            for j in range(5):
                if j < 4:
                    emit_S(j)
                if fillers is not None:
                    for f in fillers(j):
                        f()
                if j >= 1:
                    emit_AV(j - 1)

        # ---- interleaved projection + LN stats ----
        def proj_stats(w_sb, bname, srcs, resid, pp, ssum, ssq, sbp):
            """4 output chunks: proj matmuls + li/sq evac + interleaved
            stats matmuls (lagged one chunk so vector keeps ahead)."""
            li_l, sq_l = [], []

            def stat_mm(m):
                nc.tensor.matmul(ssum[:], lhsT=ones128[:], rhs=li_l[m][:],
                                 start=(m == 0), stop=(m == DC - 1),
                                 skip_group_check=True)
                nc.tensor.matmul(ssq[:], lhsT=ones128[:], rhs=sq_l[m][:],
                                 start=(m == 0), stop=(m == DC - 1),
                                 skip_group_check=True)

            for m in range(DC):
                ps = pp.tile([P, TOWN], F32, tag="proj")
                for c in range(DC):
                    nc.tensor.matmul(
                        ps[:], lhsT=w_sb[:, c, ts(m, P)], rhs=srcs[c][:],
                        start=(c == 0), stop=(c == DC - 1),
                        skip_group_check=True)
                li = sbp.tile([P, TOWN], MF, tag=f"li{m}", name=f"li{m}",
                              bufs=1)
                nc.vector.scalar_tensor_tensor(
                    li[:], in0=ps[:], scalar=col(bname, m),
                    in1=resid[m][:], op0=OP.add, op1=OP.add)
                sq = sbp.tile([P, TOWN], MF, tag=f"sq{m}", name=f"sq{m}",
                              bufs=1)
                nc.scalar.activation(sq[:], li[:], AF.Square)
                li_l.append(li)
                sq_l.append(sq)
                if m >= 1:
                    stat_mm(m - 1)
            stat_mm(DC - 1)
            return li_l

        def stats_chain(ssum, ssq, sbp):
            """ssum/ssq PSUM -> (mu bf16, rstd f32), both [P,TOWN].

            rstd = exp(-0.5*ln(var+eps)) keeps the scalar engine on the
            ln/exp activation table (no table swap around attention's Exp).
            """
            mu = sbp.tile([P, TOWN], MF, tag="mu", bufs=1)
            nc.vector.tensor_scalar(mu[:], ssum[:], 1.0 / D, None,
                                    op0=OP.mult)
            musq = sbp.tile([P, TOWN], F32, tag="musq", bufs=1)
            nc.vector.tensor_tensor(musq[:], mu[:], mu[:], op=OP.mult)
            var = sbp.tile([P, TOWN], F32, tag="var", bufs=1)
            nc.vector.scalar_tensor_tensor(
                var[:], in0=ssq[:], scalar=1.0 / D, in1=musq[:],
                op0=OP.mult, op1=OP.subtract)
            lnv = sbp.tile([P, TOWN], F32, tag="lnv", bufs=1)
            nc.scalar.activation(lnv[:], var[:], AF.Ln, bias=eps_p[:],
                                 scale=1.0)
            rstd = sbp.tile([P, TOWN], F32, tag="rstd", bufs=1)
            nc.scalar.activation(rstd[:], lnv[:], AF.Exp, scale=-0.5)
            return mu, rstd

        def norm_chunk(dst, src, mu, rstd, gcol, bcol, sbp):
            t1 = sbp.tile([P, TOWN], F32, tag="lnt", bufs=2)
            nc.vector.tensor_tensor(t1[:], src[:], mu[:], op=OP.subtract)
            t2 = sbp.tile([P, TOWN], MF, tag="lnt2", bufs=2)
            nc.vector.tensor_tensor(t2[:], t1[:], rstd[:], op=OP.mult)
            nc.vector.tensor_scalar(dst[:], t2[:], gcol, bcol,
                                    op0=OP.mult, op1=OP.add)

        # ====== blocks A+B ======
        with tc.tile_pool(name="blkB", bufs=1) as bB:
            ekT = [bB.tile([P, T], MF, tag=f"ekT{c}", name=f"ekT{c}")
                   for c in range(DC)]
            evsb = [bB.tile([P, H * P], MF, tag=f"ev{t}", name=f"ev{t}")
                    for t in range(8)]
            cqT = [bB.tile([P, TOWN], MF, tag=f"cqT{c}", name=f"cqT{c}")
                   for c in range(DC)]
            mergedT2 = [bB.tile([P, TOWN], MF, tag=f"mg2T{c}", name=f"mg2T{c}")
                        for c in range(DC)]

            with tc.tile_pool(name="blkA", bufs=1) as bA:
                x0p = [bA.tile([P, TOWN], MF, tag=f"x0p{c}", name=f"x0p{c}")
                       for c in range(DC)]
                x0o = [bA.tile([P, TOWN], MF, tag=f"x0o{c}", name=f"x0o{c}")
                       for c in range(DC)]
                kT = [bA.tile([P, T], MF, tag=f"kT{c}", name=f"kT{c}")
                      for c in range(DC)]
                vsb = [bA.tile([P, H * P], MF, tag=f"v{t}", name=f"v{t}")
                       for t in range(8)]
                qT = [bA.tile([P, TOWN], MF, tag=f"qT{c}", name=f"qT{c}")
                      for c in range(DC)]
                mergedT = [bA.tile([P, TOWN], MF, tag=f"mgT{c}",
                                   name=f"mgT{c}") for c in range(DC)]

                def x0slice(c, t):
                    return (x0p[c][:, ts(t, P)] if t < 4
                            else x0o[c][:, ts(t - 4, P)])

                # ---- early phase: tensors that die before attention ----
                with tc.tile_pool(name="early", bufs=1) as ep:
                    # DMAs in dependency order: encT/ewk feed the first
                    # real tensor work, then idx releases the gathers
                    encT_sb = ep.tile([P, DC, T], MF, tag="encT")
                    nc.sync.dma_start(
                        encT_sb[:], a["encT"].rearrange("c p t -> p c t"))
                    ewk_sb = wload(ep, "eWkT", DC, D)
                    ewv_sb = wload(ep, "eWvT", DC, D)
                    nc.sync.dma_start(
                        idx_sb[:], a["idx"].rearrange("(c p) o -> p (c o)",
                                                      p=P))
                    # PE warmup: dummy matmuls ramp the clock while DMAs land
                    nc.vector.memset(warm[:], 0.0)
                    with tc.tile_pool(name="psWarm", bufs=1,
                                      space="PSUM") as wpp:
                        wps = wpp.tile([P, TOWN], F32, tag="wps")
                        for _ in range(NWARM):
                            nc.tensor.matmul(wps[:], lhsT=warm[:, 0:P],
                                             rhs=warm[:],
                                             start=True, stop=True)
                    # pre-fill the denominator-mask half of all V tiles
                    # (vector is idle here; drops one op per vproj_unit)
                    wz = warm[:].rearrange("p (h e) -> p h e", e=DKH)
                    for t in range(8):
                        for vt, mnm in ((vsb[t], "mvS"), (evsb[t], "mvC")):
                            v3 = vt[:].rearrange("p (h e) -> p h e", e=P)
                            nc.vector.tensor_scalar(
                                v3[:, :, DKH:P], wz, 0.0, col(mnm, t),
                                op0=OP.mult, op1=OP.add)
                    # embedding gathers start as soon as idx lands
                    xg = [ep.tile([P, D], BF16, tag=f"xg{t}", name=f"xg{t}")
                          for t in range(8)]
                    for t in range(8):
                        nc.gpsimd.indirect_dma_start(
                            out=xg[t][:], out_offset=None, in_=a["emb"][:, :],
                            in_offset=bass.IndirectOffsetOnAxis(
                                ap=idx_sb[:, t: t + 1], axis=0))
                    wk_sb = wload(bA, "WkT", DC, D)
                    wq_sb = wload(bA, "WqT", DC, D)
                    wv_sb = wload(bA, "WvT", DC, D)
                    peT_sb = ep.tile([P, DC, T], BF16, tag="peT")
                    nc.sync.dma_start(
                        peT_sb[:], a["peT"].rearrange("c p t -> p c t"))
                    nc.sync.dma_start(tri[:], a["tri"][:, :])

                    # device constants (gpsimd iota queued after gathers)
                    identf = ep.tile([P, P], F32, tag="identf")
                    make_identity(nc, identf[:])
                    ident_b = cp.tile([P, P], BF16, tag="ident_b")
                    nc.scalar.copy(ident_b[:], identf[:])
                    zscr = ep.tile([P, P], F32, tag="zscr")
                    nc.vector.memset(zscr[:], 0.0)
                    ones128 = cp.tile([P, P], MF, tag="ones128")
                    nc.scalar.add(ones128[:], zscr[:], 1.0)
                    eps_p = cp.tile([P, 1], F32, tag="eps_p")
                    nc.vector.memset(eps_p[:], 1e-5)

                    def enc_slice(c, t):
                        return encT_sb[:, c, ts(t, P)]

                    # early tensor work: cross-attn K/V while gathers land
                    with tc.tile_pool(name="psEk", bufs=3,
                                      space="PSUM") as ekp:
                        for m in range(DC):
                            for th in range(2):
                                ps = ekp.tile([P, 512], F32, tag="ek")
                                for c in range(DC):
                                    nc.tensor.matmul(
                                        ps[:], lhsT=ewk_sb[:, c, ts(m, P)],
                                        rhs=encT_sb[:, c,
                                                    th * 512:(th + 1) * 512],
                                        start=(c == 0), stop=(c == DC - 1))
                                nc.scalar.activation(
                                    ekT[m][:, th * 512:(th + 1) * 512],
                                    ps[:], AF.Identity, bias=col("ebk", m))
                        for t in range(8):
                            vproj_unit(evsb[t], enc_slice, ewv_sb,
                                       col("mvC", t), ekp, "ek", t)

                    # --- embedding transpose + scale + pos encoding ---
                    with tc.tile_pool(name="psA0", bufs=4,
                                      space="PSUM") as pp0:
                        for t in range(8):
                            for c in range(DC):
                                tp = pp0.tile([P, P], BF16, tag="tp")
                                nc.tensor.transpose(tp[:], xg[t][:, ts(c, P)],
                                                    ident_b[:])
                                nc.vector.scalar_tensor_tensor(
                                    x0slice(c, t), in0=tp[:], scalar=SQRT_D,
                                    in1=peT_sb[:, c, ts(t, P)],
                                    op0=OP.mult, op1=OP.add)

                # --- K, Q projections (V deferred into attention fillers) ---
                with tc.tile_pool(name="psA1", bufs=6, space="PSUM") as pp1:
                    proj_fm(kT, [x0p, x0o], wk_sb, col("bk", 0, DC), pp=pp1)
                    proj_fm(qT, [x0o], wq_sb, col("bq", 0, DC), pp=pp1)

                # --- self-attention; fillers: vsb units then weight DMAs ---
                with tc.tile_pool(name="psS", bufs=1, space="PSUM") as sp, \
                     tc.tile_pool(name="psAV", bufs=1, space="PSUM") as avp, \
                     tc.tile_pool(name="psV", bufs=2, space="PSUM") as vp, \
                     tc.tile_pool(name="sbA", bufs=1) as sbp:

                    wlate = {}
                    filler_items = []
                    for t in range(8):
                        def f_v(t=t):
                            vproj_unit(vsb[t], x0slice, wv_sb,
                                       col("mvS", t), vp, "vproj", t)
                        filler_items.append(f_v)
                    for nm, nch, width in (("Wo1T", DC, D), ("cWqT", DC, D),
                                           ("Wo2T", DC, D), ("W1T", DC, FFN),
                                           ("W2T", FC, D)):
                        def f_w(nm=nm, nch=nch, width=width):
                            wlate[nm] = wload(wlp, nm, nch, width)
                        filler_items.append(f_w)
                    for vg in range(2):
                        def f_wo(vg=vg):
                            load_wout(vg)
                        filler_items.append(f_wo)

                    def fillers(j):
                        # all 8 vsb units must land before emit_AV(0) at j=1
                        cuts = [0, 4, 8, 11, 13, 15]
                        return filler_items[cuts[j]:cuts[j + 1]]

                    attention(kT, vsb, qT, True, mergedT,
                              (sp, avp, sbp), s_bufs=2, av_bufs=2,
                              fillers=fillers)
                    # tail dummies: keep the clock up while the last AV's
                    # vector chain and exp's psum reads drain, so Wo1
                    # starts at full speed
                    for i in range(16):
                        dt_ = vp.tile([P, D], F32, tag="vproj",
                                      name=f"dumA{i}")
                        nc.tensor.matmul(dt_[:], lhsT=warm[:, 0:P],
                                         rhs=warm[:], start=True, stop=True)

                # --- Wo1 + residual + LN1 stats (interleaved) ---
                with tc.tile_pool(name="psA2", bufs=2, space="PSUM") as pp2, \
                     tc.tile_pool(name="psBC", bufs=1, space="PSUM") as bcp, \
                     tc.tile_pool(name="sbLN1", bufs=1) as sbp:
                    ssum1 = bcp.tile([P, TOWN], F32, tag="ssum")
                    ssq1 = bcp.tile([P, TOWN], F32, tag="ssq")
                    li1 = proj_stats(wlate["Wo1T"], "bo1", mergedT, x0o,
                                     pp2, ssum1, ssq1, sbp)
                    mu1, rstd1 = stats_chain(ssum1, ssq1, sbp)

                    # cq matmuls on pre-norm li1 (g1 folded into cWqT on host)
                    with tc.tile_pool(name="psCQ", bufs=1,
                                      space="PSUM") as cqp:
                        cps = [cqp.tile([P, TOWN], F32, tag=f"cq{m}",
                                        name=f"cq{m}") for m in range(DC)]
                        for c in range(DC):
                            for m in range(DC):
                                nc.tensor.matmul(
                                    cps[m][:],
                                    lhsT=wlate["cWqT"][:, c, ts(m, P)],
                                    rhs=li1[c][:],
                                    start=(c == 0), stop=False,
                                    skip_group_check=True)
                        # rank-1 -mu*s1 correction folded into the matmul
                        # accumulation (K=1 row), so the evac is one tt + one
                        # scalar op: cq = raw*rstd + cbq
                        for m in range(DC):
                            nc.tensor.matmul(
                                cps[m][:], lhsT=srows[0:1, ts(m, P)],
                                rhs=mu1[0:1, :],
                                start=False, stop=True,
                                skip_group_check=True)
                        for m in range(DC):
                            t2 = sbp.tile([P, TOWN], F32, tag="cqt2", bufs=2)
                            nc.vector.tensor_tensor(t2[:], cps[m][:],
                                                    rstd1[:], op=OP.mult)
                            nc.scalar.activation(cqT[m][:], t2[:], AF.Identity,
                                                 bias=col("cbq", m))
                    # x1T (normalized) deferred: used only as Wo2 residual
                    for c in range(DC):
                        norm_chunk(x1T[c], li1[c], mu1, rstd1,
                                   col("gc1", c), col("bc1", c), sbp)

            # --- cross-attention ---
            # exp-bound: inject dummy matmuls in the filler slots to hold
            # the PE's DVFS clock at full speed through the exp stalls.
            with tc.tile_pool(name="psDum", bufs=1, space="PSUM") as dup, \
                 tc.tile_pool(name="psS", bufs=1, space="PSUM") as sp, \
                 tc.tile_pool(name="psAV", bufs=1, space="PSUM") as avp, \
                 tc.tile_pool(name="sbB", bufs=1) as sbp:
                dps = dup.tile([P, TOWN], F32, tag="dum")
                # hold the clock while the LN1 chain produces cqT[0]
                for _ in range(10):
                    nc.tensor.matmul(dps[:], lhsT=warm[:, 0:P], rhs=warm[:],
                                     start=True, stop=True)

                def cross_fillers(j):
                    def f():
                        for _ in range(12):
                            nc.tensor.matmul(dps[:], lhsT=warm[:, 0:P],
                                             rhs=warm[:],
                                             start=True, stop=True)
                    return [f] if j >= 1 else []

                attention(ekT, evsb, cqT, False, mergedT2, (sp, avp, sbp),
                          s_bufs=2, fillers=cross_fillers)
                # tail dummies bridge into the Wo2 block at full clock
                for _ in range(8):
                    nc.tensor.matmul(dps[:], lhsT=warm[:, 0:P], rhs=warm[:],
                                     start=True, stop=True)

            # --- Wo2 + residual + LN2 stats (interleaved) ---
            # li2/mu2/rstd2 are consumed by the FFN after blkB closes, so
            # they live in the kernel-wide trunk pool.
            with tc.tile_pool(name="psB2", bufs=2, space="PSUM") as pp2, \
                 tc.tile_pool(name="psBC", bufs=1, space="PSUM") as bcp:
                ssum2 = bcp.tile([P, TOWN], F32, tag="ssum")
                ssq2 = bcp.tile([P, TOWN], F32, tag="ssq")
                li2 = proj_stats(wlate["Wo2T"], "bo2", mergedT2, x1T,
                                 pp2, ssum2, ssq2, trunkp)
                mu2, rstd2 = stats_chain(ssum2, ssq2, trunkp)

        # ================= blocks C+D =================
        with tc.tile_pool(name="late", bufs=1) as latep, \
             tc.tile_pool(name="sbC", bufs=1) as sbp:
            xpT = [latep.tile([P, TOWN], MF, tag=f"xpT{c}", name=f"xpT{c}")
                   for c in range(DC)]
            hT = [latep.tile([P, TOWN], MF, tag=f"hT{m}", name=f"hT{m}")
                  for m in range(FC)]
            mu3s = sbp.tile([1, TOWN], F32, tag="mu3s")
            rstd3s = sbp.tile([1, TOWN], F32, tag="rstd3s")

            # ----- FFN on pre-norm li2 (g3 folded into W1T on host) -----
            # psY opens first so W2's accumulators get banks disjoint from
            # psH: W2 matmuls then never wait on wave-B evacuation drains.
            with tc.tile_pool(name="psY", bufs=1, space="PSUM") as yp:
              yps = [yp.tile([P, TOWN], F32, tag=f"y{m}", name=f"y{m}")
                     for m in range(DC)]
              with tc.tile_pool(name="psH", bufs=1, space="PSUM") as hp:
                for wave in range(2):
                    hps = [hp.tile([P, TOWN], F32, tag=f"h{m}",
                                   name=f"h{wave}_{m}") for m in range(4)]
                    # m-major with the rank-1 -mu*ws1 correction as a K=1
                    # matmul row: hps[m] stops after 5 matmuls and the evac
                    # is a single tt (+ scalar relu)
                    for m in range(4):
                        gm = wave * 4 + m
                        for c in range(DC):
                            nc.tensor.matmul(
                                hps[m][:],
                                lhsT=wlate["W1T"][:, c, ts(gm, P)],
                                rhs=li2[c][:],
                                start=(c == 0), stop=False,
                                skip_group_check=True)
                        nc.tensor.matmul(
                            hps[m][:], lhsT=srows[0:1, D + gm * P:
                                                   D + (gm + 1) * P],
                            rhs=mu2[0:1, :],
                            start=False, stop=True,
                            skip_group_check=True)
                    for m in range(4):
                        gm = wave * 4 + m
                        # h = relu(raw*rstd + b1)
                        t2 = sbp.tile([P, TOWN], F32, tag="ht2", bufs=2)
                        nc.vector.tensor_tensor(t2[:], hps[m][:], rstd2[:],
                                                op=OP.mult)
                        nc.scalar.activation(hT[gm][:], t2[:], AF.Relu,
                                             bias=col("b1", gm))
                    if wave == 0:
                        # x2T (normalized) deferred: used only as W2 residual
                        for c in range(DC):
                            norm_chunk(x2T[c], li2[c], mu2, rstd2,
                                       col("gc2", c), col("bc2", c), sbp)
                    else:
                        # W2 first half (wave-A inputs) interleaves here so
                        # the tensor queue never drains at the wave boundary
                        for hc in range(4):
                            for m in range(DC):
                                nc.tensor.matmul(
                                    yps[m][:],
                                    lhsT=wlate["W2T"][:, hc, ts(m, P)],
                                    rhs=hT[hc][:],
                                    start=(hc == 0), stop=False,
                                    skip_group_check=True)

              # W2 second half; then x_pre + LN3 stats
              with tc.tile_pool(name="psBC3", bufs=1, space="PSUM") as bcp3:
                # hold the clock while wave-B evacuations produce hT[4..7]
                with tc.tile_pool(name="psDum2", bufs=1, space="PSUM") as dp2:
                    dps2 = dp2.tile([P, TOWN], F32, tag="dum2")
                    for _ in range(12):
                        nc.tensor.matmul(dps2[:], lhsT=warm[:, 0:P],
                                         rhs=warm[:], start=True, stop=True)
                for hc in range(4, FC):
                    for m in range(DC):
                        nc.tensor.matmul(
                            yps[m][:], lhsT=wlate["W2T"][:, hc, ts(m, P)],
                            rhs=hT[hc][:],
                            start=False, stop=(hc == FC - 1),
                            skip_group_check=True)
                # hold the clock across the xpT/sq3 chain into the vocab
                with tc.tile_pool(name="psDum3", bufs=1, space="PSUM") as dp3:
                    dps3 = dp3.tile([P, TOWN], F32, tag="dum3")
                    for _ in range(8):
                        nc.tensor.matmul(dps3[:], lhsT=warm[:, 0:P],
                                         rhs=warm[:], start=True, stop=True)
                # x_pre = W2 out + b2 + x2n ; LN3 stats interleaved
                if True:
                    ssum3 = bcp3.tile([P, TOWN], F32, tag="ssum")
                    ssq3 = bcp3.tile([P, TOWN], F32, tag="ssq")
                    sq3 = []
                    for m in range(DC):
                        nc.vector.scalar_tensor_tensor(
                            xpT[m][:], in0=yps[m][:], scalar=col("b2", m),
                            in1=x2T[m][:], op0=OP.add, op1=OP.add)
                        sq = sbp.tile([P, TOWN], MF, tag=f"sq3{m}",
                                      name=f"sq3{m}", bufs=1)
                        nc.scalar.activation(sq[:], xpT[m][:], AF.Square)
                        sq3.append(sq)
                    for m in range(DC):
                        nc.tensor.matmul(ssum3[:], lhsT=ones128[:],
                                         rhs=xpT[m][:],
                                         start=(m == 0), stop=(m == DC - 1),
                                         skip_group_check=True)
                        nc.tensor.matmul(ssq3[:], lhsT=ones128[:],
                                         rhs=sq3[m][:],
                                         start=(m == 0), stop=(m == DC - 1),
                                         skip_group_check=True)
                    mu3, rstd3 = stats_chain(ssum3, ssq3, sbp)
                    nc.vector.tensor_copy(mu3s[:], mu3[0:1, :])
                    nc.vector.tensor_copy(rstd3s[:], rstd3[0:1, :])
                    nc.sync.dma_start(a["stats"][0:1, :], mu3s[:])
                    nc.sync.dma_start(a["stats"][1:2, :], rstd3s[:])

            # ===== block D: vocab projection on pre-norm x_pre =====
            # (g2 folded into WoutT on host; mu/rstd dequant on host)
            with tc.tile_pool(name="stD", bufs=4) as stp, \
                 tc.tile_pool(name="psD", bufs=2, space="PSUM") as pp:
                for vg in range(NVG):
                    if vg + 2 < NVG:
                        load_wout(vg + 2)
                    w_sb = wD_tiles.pop(vg)
                    for t in range(TOWN // P):
                        ps = pp.tile([P, VG, 512], F32, tag="vps")
                        for j in range(VG):
                            for c in range(DC):
                                nc.tensor.matmul(
                                    ps[:, j, 0:VCH],
                                    lhsT=xpT[c][:, ts(t, P)],
                                    rhs=w_sb[:, c, ts(j, VCH)],
                                    start=(c == 0), stop=(c == DC - 1))
                        stage = stp.tile([P, GW], OUT_DT, tag="stage")
                        st3 = stage[:].rearrange("p (j e) -> p j e", e=VCH)
                        if t % 2 == 0:
                            nc.scalar.copy(st3, ps[:, :, 0:VCH])
                        else:
                            nc.vector.tensor_copy(st3, ps[:, :, 0:VCH])
                        nc.sync.dma_start(
                            a["out"][ts(t, P), vg * GW:(vg + 1) * GW],
                            stage[:])


# --------------------------------------------------------------------------
# host-side input preparation
# --------------------------------------------------------------------------

def _pos_encoding_np(t, d):
    pos = np.arange(t, dtype=np.float32)[:, None]
    freqs = 1.0 / (10000.0 ** (np.arange(0, d, 2, dtype=np.float32) / d))
    pe = np.zeros((t, d), np.float32)
    pe[:, 0::2] = np.sin(pos * freqs)
    pe[:, 1::2] = np.cos(pos * freqs)
    return pe


def _col_pack(b):
    b = np.asarray(b, np.float32)
    return np.ascontiguousarray(b.reshape(-1, P).T)


def prep_in_maps(inputs):
    import ml_dtypes
    BF = ml_dtypes.bfloat16
    gi = lambda n: np.asarray(inputs[n], np.float32)
    tokens = np.asarray(inputs["tokens"]).astype(np.int32)
    enc_all = np.ascontiguousarray(gi("enc_embeddings"))
    enc_pad = np.asarray(inputs["enc_pad_mask"]).astype(bool)
    emb = np.ascontiguousarray(gi("emb").astype(BF))

    shared = {"emb": emb}
    for nm in ("Wq", "Wk", "Wv", "Wo1", "Wo2", "eWk", "eWv", "W2"):
        shared[nm + "T"] = np.ascontiguousarray(gi(nm).T).astype(BF)
    # LN gain folds: g1 into cWq, g3 into W1, g2 into Wout
    cWq_f = gi("cWq") * gi("g1")[None, :]
    shared["cWqT"] = np.ascontiguousarray(cWq_f.T).astype(BF)
    W1_f = gi("W1") * gi("g3")[None, :]
    shared["W1T"] = np.ascontiguousarray(W1_f.T).astype(BF)
    Wout_f = gi("Wout") * gi("g2")[None, :]
    shared["WoutT"] = np.ascontiguousarray(Wout_f.T).astype(BF)

    kk = np.arange(P)[:, None]
    qq = np.arange(P)[None, :]
    shared["tri"] = np.where(kk > qq, NEG, 0.0).astype(BF)

    base_cols = np.zeros((P, NCOLS), np.float32)

    def put(nm, arr):
        off, w = _COL_LAYOUT[nm]
        base_cols[:, off:off + w] = _col_pack(arr)
    # V-bias folds: bv -> bo1, ebv -> bo2 (exact: softmax weights sum to 1)
    put("bq", gi("bq")); put("bk", gi("bk"))
    put("bo1", gi("bo1") + gi("Wo1") @ gi("bv"))
    put("ebk", gi("ebk"))
    put("bo2", gi("bo2") + gi("Wo2") @ gi("ebv"))
    put("b2", gi("b2"))
    # LN-fold constant terms: consumer bias absorbs W @ beta
    put("cbq", gi("cbq") + gi("cWq") @ gi("be1"))
    put("b1", gi("b1") + gi("W1") @ gi("be3"))
    put("gc1", gi("g1")); put("bc1", gi("be1"))
    put("gc2", gi("g3")); put("bc2", gi("be3"))
    # LN-fold correction columns: -(W*g).sum over input dim, per out feature
    put("cs1", -cWq_f.sum(axis=1))
    put("ws1", -W1_f.sum(axis=1))
    # same corrections as rows, consumed by K=1 rank-1 matmuls
    shared["srows"] = np.concatenate(
        [-cWq_f.sum(axis=1), -W1_f.sum(axis=1)]
    ).reshape(1, D + FFN).astype(BF)

    pe = _pos_encoding_np(T, D)

    in_maps = []
    for core in range(8):
        b, hf = core // 2, core % 2
        own = tokens[b, hf * 512:(hf + 1) * 512]
        idx_full = np.concatenate([tokens[b, :512], own])
        pe_slots = np.concatenate([pe[:512], pe[hf * 512:(hf + 1) * 512]],
                                  axis=0)
        peT = np.ascontiguousarray(
            pe_slots.T.reshape(DC, P, T, order="C")).astype(BF)
        mvS = (idx_full != PAD_ID).astype(np.float32)
        if hf == 0:
            mvS[:512] = 0.0
        mvC = np.where(enc_pad[b], 0.0, 1.0).astype(np.float32)
        encT = np.ascontiguousarray(
            enc_all[b].T.reshape(DC, P, T)).astype(BF)
        cols = base_cols.copy()
        offS, _ = _COL_LAYOUT["mvS"]
        offC, _ = _COL_LAYOUT["mvC"]
        cols[:, offS:offS + 8] = mvS.reshape(8, P).T
        cols[:, offC:offC + 8] = mvC.reshape(8, P).T
        m = dict(shared)
        m["idx"] = np.ascontiguousarray(idx_full.reshape(T, 1))
        m["peT"] = peT
        m["encT"] = encT
        m["cols"] = np.ascontiguousarray(cols)
        in_maps.append(m)
    return in_maps


def assemble(results, inputs):
    gi = lambda n: np.asarray(inputs[n], np.float32)
    Wout = gi("Wout")
    # device computed raw = x_pre @ (Wout*g2).T ; finish LN3 affine here:
    # out = rstd*raw - (rstd*mu)*s1 + c2, s1 = Wout@g2, c2 = Wout@be2 + bout
    s1 = Wout @ gi("g2")
    c2 = Wout @ gi("be2") + gi("bout")
    full = np.empty((4, 1024, V), np.float32)
    for core in range(8):
        b, hf = core // 2, core % 2
        raw = np.asarray(results[core]["out"]).astype(np.float32)
        st = np.asarray(results[core]["stats"]).astype(np.float32)
        mu, rstd = st[0], st[1]
        full[b, hf * 512:(hf + 1) * 512] = (
            rstd[:, None] * raw
            + (-(rstd * mu))[:, None] * s1[None, :]
            + c2[None, :])
    return full


def kernel(**inputs):
    from concourse.bass_utils import run_bass_kernel_spmd
    nc = build_module()
    in_maps = prep_in_maps(inputs)
    res = run_bass_kernel_spmd(nc, in_maps, core_ids=list(range(8)))
    return assemble(res.results, inputs)


if __name__ == "__main__":
    nc = build_module()
    print("built ok")
